# revision 3
# baseline (speedup 1.0000x reference)
"""BitLinear (W1.58 / int8-activation quant) dense layer on 8 Trainium2 cores.

Math (matches the reference exactly up to fp rounding):
  alpha = mean(|W|) + 1e-8                     (global absmean, ternary weight scale)
  Wq    = clip(round(W/alpha), -1, 1)          (ternary, exact in bf16)
  gamma = clip(max|x_row|, 1e-8)               (per-token absmax)
  Qx    = round(x * 127/gamma)                 (integers in [-127,127], exact in bf16)
  out   = (Qx @ Wq.T) * (gamma * alpha / 127)

The integer matmul (bf16 operands, fp32 PSUM accumulation) is exact: products
are integers <= 127 and partial sums < 2^24.  All quantization, rounding and
the matmul run on device; the host only shards/reassembles and adds the eight
per-core |W| partial sums (launch A) into the scalar alpha.

Sharding: column-parallel (tensor-parallel on fan_out).  Each core owns a
2048-wide slice of the 16384 fan_out, x is replicated, no collectives.
"""

import sys
import types

import numpy as np

for _p in ("/opt/trn_rl_repo",):
    if _p not in sys.path:
        sys.path.insert(0, _p)

import concourse.bass as bass  # noqa: E402
import concourse.mybir as mybir  # noqa: E402
import concourse.tile as tile  # noqa: E402
from concourse import bacc  # noqa: E402
from concourse.bass_utils import run_bass_kernel_spmd  # noqa: E402

F32 = mybir.dt.float32
BF16 = mybir.dt.bfloat16
AX = mybir.AxisListType.X
ALU = mybir.AluOpType
ACT_FN = mybir.ActivationFunctionType

N_CORES = 8
FAN_IN = 4096
FAN_OUT = 16384
NTOK = 4 * 2048                     # 8192 tokens
FO_CORE = FAN_OUT // N_CORES        # 2048 fan_out columns per core
MBLK = 128                          # token block (PE partition dim)
N_MBLK = NTOK // MBLK               # 64 token blocks
KT = FAN_IN // 128                  # 32 contraction tiles of 128
NCH = FO_CORE // 512                # 4 psum chunks of 512
MAGIC = 12582912.0                  # 1.5 * 2**23: (v+MAGIC)-MAGIC == rint(v)


def _install_ntff_hook():
    """Register the axon NTFF profile hook trn_boot couldn't install
    (this image's antenv package lacks the axon_hooks submodule)."""
    if "antenv.axon_hooks" in sys.modules:
        return
    try:
        from trn_agent_boot.trn_boot import _ntff_profile_via_ctypes

        hook = _ntff_profile_via_ctypes("/opt/axon/libaxon_pjrt.so")
    except Exception:
        hook = None
    mod = types.ModuleType("antenv.axon_hooks")
    mod.get_axon_ntff_profile_hook = lambda: hook
    mod.set_axon_ntff_profile_hook = lambda h: None
    sys.modules["antenv.axon_hooks"] = mod
    import antenv

    antenv.axon_hooks = mod


def build_alpha_nc():
    """Launch A: per-core partial sum of |W| (per partition), for alpha."""
    nc = bacc.Bacc("TRN2", target_bir_lowering=False, debug=False,
                   num_devices=N_CORES)
    n_t = FO_CORE // MBLK  # 16 tiles of [128, 4096]
    w_in = nc.declare_dram_parameter("w", [n_t, MBLK, FAN_IN], F32, isOutput=False)
    s_out = nc.declare_dram_parameter("s", [MBLK, 1], F32, isOutput=True)
    with tile.TileContext(nc) as tc:
        with tc.tile_pool(name="io", bufs=3) as pool, \
             tc.tile_pool(name="acc", bufs=1) as accp:
            accv = accp.tile([MBLK, n_t], F32)
            for i in range(n_t):
                t = pool.tile([MBLK, FAN_IN], F32)
                nc.sync.dma_start(t[:], w_in[i])
                nc.vector.reduce_sum(accv[:, i:i + 1], t[:], axis=AX,
                                     apply_absolute_value=True)
            total = accp.tile([MBLK, 1], F32)
            nc.vector.reduce_sum(total[:], accv[:], axis=AX)
            nc.sync.dma_start(s_out[:], total[:])
    nc.compile()
    return nc


def build_main_nc(n_mblk=N_MBLK, nmm=512):
    """Launch B: quantize W (ternary) + x (int8 grid), exact bf16 matmul,
    per-token rescale.  One core's fan_out slice, all tokens.

    Schedule notes:
    - x loads + qx transposes ride the ACT HWDGE ring; the 32 MiB weight
      stream owns the SP ring so it is never stalled behind an x-tile
      WAR wait.
    - The x pipelines for m=0,1 are emitted before the weight stream so
      their DMAs/quant run during the weight DMA (PE warm-up food).
    - W-quant is a 3-engine pipeline (ACT round / DVE min / GpSimd max)
      so DVE keeps serving the x pipeline during the ramp.
    - The first two token blocks share one a-loop (8 PSUM banks) so the
      PE has 2 blocks of matmul work while weight tiles trickle in.
    """
    assert FO_CORE % nmm == 0
    nch = FO_CORE // nmm
    banks = nmm // 512
    nc = bacc.Bacc("TRN2", target_bir_lowering=False, debug=False,
                   num_devices=N_CORES)
    x_in = nc.declare_dram_parameter("x", [n_mblk, MBLK, FAN_IN], F32,
                                     isOutput=False)
    # wg[a, p, n] = W_slice.T[a*128 + p, n]  (k-tile a, k-within-tile p)
    wg_in = nc.declare_dram_parameter("wg", [KT, 128, FO_CORE], F32,
                                      isOutput=False)
    ia_in = nc.declare_dram_parameter("inv_alpha", [128, 1], F32, isOutput=False)
    a127_in = nc.declare_dram_parameter("alpha127", [128, 1], F32, isOutput=False)
    out_d = nc.declare_dram_parameter("out", [n_mblk, MBLK, FO_CORE], F32,
                                      isOutput=True)

    with tile.TileContext(nc) as tc:
        with tc.tile_pool(name="consts", bufs=1) as cpool, \
             tc.tile_pool(name="vec", bufs=4) as vpool, \
             tc.tile_pool(name="xio", bufs=2) as xpool, \
             tc.tile_pool(name="qxp", bufs=1) as qxpool, \
             tc.tile_pool(name="qxtp", bufs=2) as qxtpool, \
             tc.tile_pool(name="osbp", bufs=2) as opool, \
             tc.tile_pool(name="wfp", bufs=2) as wfpool, \
             tc.tile_pool(name="wqp", bufs=KT) as wqpool, \
             tc.tile_pool(name="ps", bufs=8 // banks, space="PSUM") as pspool:

            magic = cpool.tile([128, 1], F32)
            nc.gpsimd.memset(magic[:], MAGIC)
            ia = cpool.tile([128, 1], F32)
            nc.sync.dma_start(ia[:], ia_in[:])
            a127 = cpool.tile([128, 1], F32)
            nc.sync.dma_start(a127[:], a127_in[:])

            def x_pipeline(m):
                """DMA+quantize+transpose one token block; returns (qxT, sc)."""
                xh = []
                g2 = vpool.tile([128, 2], F32, tag="g2", name="g2")
                for h in range(2):
                    xt = xpool.tile([128, FAN_IN // 2], F32, tag="xt", name="xt")
                    nc.scalar.dma_start(xt[:], x_in[m, :, h * 2048:(h + 1) * 2048])
                    nc.vector.reduce_max(g2[:, h:h + 1], xt[:], axis=AX,
                                         apply_absolute_value=True)
                    xh.append(xt)
                g = vpool.tile([128, 1], F32, tag="g", name="g")
                nc.vector.reduce_max(g[:], g2[:], axis=AX)
                nc.vector.tensor_scalar_max(g[:], g[:], 1e-8)
                sc = vpool.tile([128, 1], F32, tag="sc", name="sc")
                nc.vector.tensor_scalar(out=sc[:], in0=g[:], scalar1=a127[:, 0:1],
                                        scalar2=None, op0=ALU.mult)
                rg = vpool.tile([128, 1], F32, tag="rg", name="rg")
                nc.vector.reciprocal(rg[:], g[:])
                nc.vector.tensor_scalar(out=rg[:], in0=rg[:], scalar1=127.0,
                                        scalar2=None, op0=ALU.mult)
                qx = qxpool.tile([128, FAN_IN], BF16, tag="qx", name="qx")
                for h in range(2):
                    # x*127/gamma + MAGIC (rounds to int), then -MAGIC -> bf16
                    nc.scalar.activation(xh[h][:], xh[h][:], ACT_FN.Identity,
                                         bias=magic[:, 0:1], scale=rg[:, 0:1])
                    nc.vector.tensor_scalar(out=qx[:, h * 2048:(h + 1) * 2048],
                                            in0=xh[h][:], scalar1=-MAGIC,
                                            scalar2=None, op0=ALU.add)
                # qxT[p, a, mm] = qx[mm, a*128 + p]
                qxT = qxtpool.tile([128, KT, 128], BF16, tag="qxT", name="qxT")
                nc.scalar.dma_start_transpose(qxT[:, :, :], qx[:, :])
                return qxT, sc

            def drain(psts, sc, m):
                """PSUM -> scaled f32 SBUF -> DRAM for one token block."""
                osb = opool.tile([128, FO_CORE], F32, tag="osb", name="osb")
                for n in range(nch):
                    nc.scalar.activation(osb[:, n * nmm:(n + 1) * nmm],
                                         psts[n][:], ACT_FN.Copy,
                                         bias=0.0, scale=sc[:, 0:1])
                nc.sync.dma_start(out_d[m], osb[:])

            # ---- x pipelines for the first two blocks (run during W DMA)
            head = [x_pipeline(m) for m in range(min(2, n_mblk))]

            # ---- Phase W: ternary-quantize the weight slice (resident bf16)
            wq_tiles = []
            for a in range(KT):
                wf = wfpool.tile([128, FO_CORE], F32, tag="wf", name="wf")
                nc.sync.dma_start(wf[:], wg_in[a])
                # wf = w*inv_alpha + MAGIC  (rounds to nearest-even integer)
                nc.scalar.activation(wf[:], wf[:], ACT_FN.Identity,
                                     bias=magic[:, 0:1], scale=ia[:, 0:1])
                # wf = min(wf - MAGIC, 1)
                nc.vector.tensor_scalar(out=wf[:], in0=wf[:], scalar1=-MAGIC,
                                        scalar2=1.0, op0=ALU.add, op1=ALU.min)
                wq = wqpool.tile([128, FO_CORE], BF16, tag="wq", name="wq")
                # wq = max(wf, -1)   -> ternary {-1, 0, 1} in bf16
                nc.gpsimd.tensor_scalar(out=wq[:], in0=wf[:], scalar1=-1.0,
                                        scalar2=None, op0=ALU.max)
                wq_tiles.append(wq)

            # ---- Ramp: first two blocks share one a-loop (PE eats 2 blocks
            #      of matmuls while wq tiles arrive)
            nhead = len(head)
            psts = [[pspool.tile([128, nmm], F32, tag="ps", name="ps")
                     for _ in range(nch)] for _ in range(nhead)]
            for a in range(KT):
                for mb in range(nhead):
                    lt = head[mb][0][:, a, :]
                    for n in range(nch):
                        nc.tensor.matmul(psts[mb][n][:], lt,
                                         wq_tiles[a][:, n * nmm:(n + 1) * nmm],
                                         start=(a == 0), stop=(a == KT - 1))
            for mb in range(nhead):
                drain(psts[mb], head[mb][1], mb)

            # ---- Steady loop
            for m in range(nhead, n_mblk):
                qxT, sc = x_pipeline(m)
                ps1 = [pspool.tile([128, nmm], F32, tag="ps", name="ps")
                       for _ in range(nch)]
                for a in range(KT):
                    lt = qxT[:, a, :]
                    for n in range(nch):
                        nc.tensor.matmul(ps1[n][:], lt,
                                         wq_tiles[a][:, n * nmm:(n + 1) * nmm],
                                         start=(a == 0), stop=(a == KT - 1))
                drain(ps1, sc, m)
    nc.compile()
    return nc


_CACHE = {}


def _get_ncs(n_mblk=N_MBLK):
    key = ("ncs", n_mblk)
    if key not in _CACHE:
        _install_ntff_hook()
        _CACHE[key] = (build_alpha_nc(), build_main_nc(n_mblk))
    return _CACHE[key]


def _shard_weight(weight):
    """Per-core host-side layout prep (pure slicing/reshape glue)."""
    wgs, was = [], []
    for c in range(N_CORES):
        w_c = weight[c * FO_CORE:(c + 1) * FO_CORE, :]          # [2048, 4096]
        was.append(np.ascontiguousarray(w_c).reshape(FO_CORE // MBLK, MBLK,
                                                     FAN_IN))
        wg = np.ascontiguousarray(w_c.T).reshape(KT, 128, FO_CORE)
        wgs.append(wg)
    return wgs, was


def run(x, weight, trace=False, n_mblk=N_MBLK):
    """Returns (out, exec_ns) where exec_ns is summed HW time of both
    launches (None unless trace=True)."""
    x = np.ascontiguousarray(np.asarray(x, dtype=np.float32))
    weight = np.ascontiguousarray(np.asarray(weight, dtype=np.float32))
    nc_a, nc_b = _get_ncs(n_mblk)
    wgs, was = _shard_weight(weight)
    core_ids = list(range(N_CORES))

    # Launch A: per-core |W| partial sums -> alpha on host (8 adds of
    # device-computed partials; the reduction itself ran on device).
    res_a = run_bass_kernel_spmd(nc_a, [{"w": was[c]} for c in core_ids],
                                 core_ids, trace=trace)
    total = np.float64(0.0)
    for c in core_ids:
        total += np.float64(res_a.results[c]["s"].sum(dtype=np.float64))
    alpha = np.float32(total / (FAN_OUT * FAN_IN)) + np.float32(1e-8)

    x3 = x.reshape(-1, FAN_IN)[: n_mblk * MBLK].reshape(n_mblk, MBLK, FAN_IN)
    ia_v = np.full((128, 1), 1.0 / alpha, dtype=np.float32)
    a127_v = np.full((128, 1), alpha / np.float32(127.0), dtype=np.float32)
    in_maps = [{"x": x3, "wg": wgs[c], "inv_alpha": ia_v, "alpha127": a127_v}
               for c in core_ids]
    res_b = run_bass_kernel_spmd(nc_b, in_maps, core_ids, trace=trace)

    outs = [res_b.results[c]["out"].reshape(n_mblk * MBLK, FO_CORE)
            for c in core_ids]
    full = np.concatenate(outs, axis=1)
    exec_ns = None
    if trace and res_a.exec_time_ns is not None and res_b.exec_time_ns is not None:
        exec_ns = res_a.exec_time_ns + res_b.exec_time_ns
    return full, exec_ns


def kernel(x, weight):
    x = np.asarray(x)
    out2d, _ = run(x, weight, trace=False)
    return out2d.reshape(*x.shape[:-1], FAN_OUT).astype(np.float32)


# revision 5
# speedup vs baseline: 1.4130x; 1.4130x over previous
"""BitLinear (W1.58 / int8-activation quant) dense layer on 8 Trainium2 cores.

Math (matches the reference exactly up to fp rounding):
  alpha = mean(|W|) + 1e-8                     (global absmean, ternary weight scale)
  Wq    = clip(round(W/alpha), -1, 1)          (ternary, exact in bf16)
  gamma = clip(max|x_row|, 1e-8)               (per-token absmax)
  Qx    = round(x * 127/gamma)                 (integers in [-127,127], exact in bf16)
  out   = (Qx @ Wq.T) * (gamma * alpha / 127)

The integer matmul (bf16 operands, fp32 PSUM accumulation) is exact: products
are integers <= 127 and partial sums < 2^24.  All quantization, rounding and
the matmul run on device; the host only shards/reassembles and adds the eight
per-core |W| partial sums (launch A) into the scalar alpha.

Sharding: column-parallel (tensor-parallel on fan_out).  Each core owns a
2048-wide slice of the 16384 fan_out, x is replicated, no collectives.
"""

import sys
import types

import numpy as np

for _p in ("/opt/trn_rl_repo",):
    if _p not in sys.path:
        sys.path.insert(0, _p)

import concourse.bass as bass  # noqa: E402
import concourse.mybir as mybir  # noqa: E402
import concourse.tile as tile  # noqa: E402
from concourse import bacc  # noqa: E402
from concourse.bass_utils import run_bass_kernel_spmd  # noqa: E402

F32 = mybir.dt.float32
BF16 = mybir.dt.bfloat16
AX = mybir.AxisListType.X
ALU = mybir.AluOpType
ACT_FN = mybir.ActivationFunctionType

N_CORES = 8
FAN_IN = 4096
FAN_OUT = 16384
NTOK = 4 * 2048                     # 8192 tokens
FO_CORE = FAN_OUT // N_CORES        # 2048 fan_out columns per core
MBLK = 128                          # token block (PE partition dim)
N_MBLK = NTOK // MBLK               # 64 token blocks
KT = FAN_IN // 128                  # 32 contraction tiles of 128
NCH = FO_CORE // 512                # 4 psum chunks of 512
MAGIC = 12582912.0                  # 1.5 * 2**23: (v+MAGIC)-MAGIC == rint(v)


def _install_ntff_hook():
    """Register the axon NTFF profile hook trn_boot couldn't install
    (this image's antenv package lacks the axon_hooks submodule)."""
    if "antenv.axon_hooks" in sys.modules:
        return
    try:
        from trn_agent_boot.trn_boot import _ntff_profile_via_ctypes

        hook = _ntff_profile_via_ctypes("/opt/axon/libaxon_pjrt.so")
    except Exception:
        hook = None
    mod = types.ModuleType("antenv.axon_hooks")
    mod.get_axon_ntff_profile_hook = lambda: hook
    mod.set_axon_ntff_profile_hook = lambda h: None
    sys.modules["antenv.axon_hooks"] = mod
    import antenv

    antenv.axon_hooks = mod


def build_alpha_nc():
    """Launch A: per-core partial sum of |W| (per partition), for alpha."""
    nc = bacc.Bacc("TRN2", target_bir_lowering=False, debug=False,
                   num_devices=N_CORES)
    n_t = FO_CORE // MBLK  # 16 tiles of [128, 4096]
    w_in = nc.declare_dram_parameter("w", [n_t, MBLK, FAN_IN], F32, isOutput=False)
    s_out = nc.declare_dram_parameter("s", [MBLK, 1], F32, isOutput=True)
    with tile.TileContext(nc) as tc:
        with tc.tile_pool(name="io", bufs=3) as pool, \
             tc.tile_pool(name="acc", bufs=1) as accp:
            accv = accp.tile([MBLK, n_t], F32)
            for i in range(n_t):
                t = pool.tile([MBLK, FAN_IN], F32)
                nc.sync.dma_start(t[:], w_in[i])
                nc.vector.reduce_sum(accv[:, i:i + 1], t[:], axis=AX,
                                     apply_absolute_value=True)
            total = accp.tile([MBLK, 1], F32)
            nc.vector.reduce_sum(total[:], accv[:], axis=AX)
            nc.sync.dma_start(s_out[:], total[:])
    nc.compile()
    return nc


def build_main_nc(n_mblk=N_MBLK, nmm=512):
    """Launch B: quantize W (ternary) + x (int8 grid), exact bf16 matmul,
    per-token rescale.  One core's fan_out slice, all tokens.

    Schedule notes:
    - x loads + qx transposes ride the ACT HWDGE ring; the 32 MiB weight
      stream owns the SP ring so it is never stalled behind an x-tile
      WAR wait.
    - The x pipelines for m=0,1 are emitted before the weight stream so
      their DMAs/quant run during the weight DMA (PE warm-up food).
    - W-quant is a 3-engine pipeline (ACT round / DVE min / GpSimd max)
      so DVE keeps serving the x pipeline during the ramp.
    - The first two token blocks share one a-loop (8 PSUM banks) so the
      PE has 2 blocks of matmul work while weight tiles trickle in.
    """
    assert FO_CORE % nmm == 0
    nch = FO_CORE // nmm
    banks = nmm // 512
    nc = bacc.Bacc("TRN2", target_bir_lowering=False, debug=False,
                   num_devices=N_CORES)
    x_in = nc.declare_dram_parameter("x", [n_mblk, MBLK, FAN_IN], F32,
                                     isOutput=False)
    # wg[a, p, n] = W_slice.T[a*128 + p, n]  (k-tile a, k-within-tile p)
    wg_in = nc.declare_dram_parameter("wg", [KT, 128, FO_CORE], F32,
                                      isOutput=False)
    ia_in = nc.declare_dram_parameter("inv_alpha", [128, 1], F32, isOutput=False)
    a127_in = nc.declare_dram_parameter("alpha127", [128, 1], F32, isOutput=False)
    out_d = nc.declare_dram_parameter("out", [n_mblk, MBLK, FO_CORE], F32,
                                      isOutput=True)

    with tile.TileContext(nc) as tc:
        with tc.tile_pool(name="consts", bufs=1) as cpool, \
             tc.tile_pool(name="vec", bufs=4) as vpool, \
             tc.tile_pool(name="xio", bufs=2) as xpool, \
             tc.tile_pool(name="qxp", bufs=1) as qxpool, \
             tc.tile_pool(name="qxtp", bufs=2) as qxtpool, \
             tc.tile_pool(name="osbp", bufs=2) as opool, \
             tc.tile_pool(name="wfp", bufs=2) as wfpool, \
             tc.tile_pool(name="wqp", bufs=KT) as wqpool, \
             tc.tile_pool(name="ps", bufs=8 // banks, space="PSUM") as pspool:

            magic = cpool.tile([128, 1], F32)
            nc.gpsimd.memset(magic[:], MAGIC)
            negmagic = cpool.tile([128, 1], F32)
            nc.gpsimd.memset(negmagic[:], -MAGIC)
            ia = cpool.tile([128, 1], F32)
            nc.sync.dma_start(ia[:], ia_in[:])
            a127 = cpool.tile([128, 1], F32)
            nc.sync.dma_start(a127[:], a127_in[:])

            def x_pipeline(m, ring=None):
                """DMA+quantize+transpose one token block; returns (qxT, sc)."""
                ring = ring or nc.sync
                xh = []
                g2 = vpool.tile([128, 2], F32, tag="g2", name="g2")
                for h in range(2):
                    xt = xpool.tile([128, FAN_IN // 2], F32, tag="xt", name="xt")
                    ring.dma_start(xt[:], x_in[m, :, h * 2048:(h + 1) * 2048])
                    nc.vector.reduce_max(g2[:, h:h + 1], xt[:], axis=AX,
                                         apply_absolute_value=True)
                    xh.append(xt)
                g = vpool.tile([128, 1], F32, tag="g", name="g")
                nc.vector.reduce_max(g[:], g2[:], axis=AX)
                nc.vector.tensor_scalar_max(g[:], g[:], 1e-8)
                sc = vpool.tile([128, 1], F32, tag="sc", name="sc")
                nc.vector.tensor_scalar(out=sc[:], in0=g[:], scalar1=a127[:, 0:1],
                                        scalar2=None, op0=ALU.mult)
                rg = vpool.tile([128, 1], F32, tag="rg", name="rg")
                nc.vector.reciprocal(rg[:], g[:])
                nc.vector.tensor_scalar(out=rg[:], in0=rg[:], scalar1=127.0,
                                        scalar2=None, op0=ALU.mult)
                qx = qxpool.tile([128, FAN_IN], BF16, tag="qx", name="qx")
                for h in range(2):
                    # x*127/gamma + MAGIC (rounds to int), then -MAGIC -> bf16
                    nc.scalar.activation(xh[h][:], xh[h][:], ACT_FN.Identity,
                                         bias=magic[:, 0:1], scale=rg[:, 0:1])
                    nc.vector.tensor_scalar(out=qx[:, h * 2048:(h + 1) * 2048],
                                            in0=xh[h][:], scalar1=-MAGIC,
                                            scalar2=None, op0=ALU.add)
                # qxT[p, a, mm] = qx[mm, a*128 + p]
                qxT = qxtpool.tile([128, KT, 128], BF16, tag="qxT", name="qxT")
                nc.scalar.dma_start_transpose(qxT[:, :, :], qx[:, :])
                return qxT, sc

            def drain(psts, sc, m):
                """PSUM -> scaled f32 SBUF -> DRAM for one token block."""
                osb = opool.tile([128, FO_CORE], F32, tag="osb", name="osb")
                for n in range(nch):
                    nc.scalar.activation(osb[:, n * nmm:(n + 1) * nmm],
                                         psts[n][:], ACT_FN.Copy,
                                         bias=0.0, scale=sc[:, 0:1])
                nc.sync.dma_start(out_d[m], osb[:])

            # ---- x pipelines for the first two blocks (run during W DMA).
            #      Block 0 loads lead the SP ring (ahead of the weight
            #      stream); block 1 loads ride the ACT ring so their WAR
            #      waits never stall the weight stream.
            head = []
            if n_mblk > 0:
                head.append(x_pipeline(0, ring=nc.sync))
            if n_mblk > 1:
                head.append(x_pipeline(1, ring=nc.scalar))

            # ---- Phase W: ternary-quantize the weight slice (resident bf16)
            wq_tiles = []
            for a in range(KT):
                wf = wfpool.tile([128, FO_CORE], F32, tag="wf", name="wf")
                nc.sync.dma_start(wf[:], wg_in[a])
                # wf = w*inv_alpha + MAGIC  (rounds to nearest-even integer)
                nc.vector.tensor_scalar(out=wf[:], in0=wf[:],
                                        scalar1=ia[:, 0:1], scalar2=MAGIC,
                                        op0=ALU.mult, op1=ALU.add)
                wq = wqpool.tile([128, FO_CORE], BF16, tag="wq", name="wq")
                # sign(rint(w/alpha)) == clip(rint(w/alpha), -1, 1): ternary
                nc.scalar.activation(wq[:], wf[:], ACT_FN.Sign,
                                     bias=negmagic[:, 0:1])
                wq_tiles.append(wq)

            # ---- Ramp: first two blocks share one a-loop (PE eats 2 blocks
            #      of matmuls while wq tiles arrive)
            nhead = len(head)
            psts = [[pspool.tile([128, nmm], F32, tag="ps", name="ps")
                     for _ in range(nch)] for _ in range(nhead)]
            for a in range(KT):
                for mb in range(nhead):
                    lt = head[mb][0][:, a, :]
                    for n in range(nch):
                        nc.tensor.matmul(psts[mb][n][:], lt,
                                         wq_tiles[a][:, n * nmm:(n + 1) * nmm],
                                         start=(a == 0), stop=(a == KT - 1))
            for mb in range(nhead):
                drain(psts[mb], head[mb][1], mb)

            # ---- Steady loop
            for m in range(nhead, n_mblk):
                qxT, sc = x_pipeline(m)
                ps1 = [pspool.tile([128, nmm], F32, tag="ps", name="ps")
                       for _ in range(nch)]
                for a in range(KT):
                    lt = qxT[:, a, :]
                    for n in range(nch):
                        nc.tensor.matmul(ps1[n][:], lt,
                                         wq_tiles[a][:, n * nmm:(n + 1) * nmm],
                                         start=(a == 0), stop=(a == KT - 1))
                drain(ps1, sc, m)
    nc.compile()
    return nc


_CACHE = {}


def _get_ncs(n_mblk=N_MBLK):
    key = ("ncs", n_mblk)
    if key not in _CACHE:
        _install_ntff_hook()
        _CACHE[key] = (build_alpha_nc(), build_main_nc(n_mblk))
    return _CACHE[key]


def _shard_weight(weight):
    """Per-core host-side layout prep (pure slicing/reshape glue)."""
    wgs, was = [], []
    for c in range(N_CORES):
        w_c = weight[c * FO_CORE:(c + 1) * FO_CORE, :]          # [2048, 4096]
        was.append(np.ascontiguousarray(w_c).reshape(FO_CORE // MBLK, MBLK,
                                                     FAN_IN))
        wg = np.ascontiguousarray(w_c.T).reshape(KT, 128, FO_CORE)
        wgs.append(wg)
    return wgs, was


def run(x, weight, trace=False, n_mblk=N_MBLK):
    """Returns (out, exec_ns) where exec_ns is summed HW time of both
    launches (None unless trace=True)."""
    x = np.ascontiguousarray(np.asarray(x, dtype=np.float32))
    weight = np.ascontiguousarray(np.asarray(weight, dtype=np.float32))
    nc_a, nc_b = _get_ncs(n_mblk)
    wgs, was = _shard_weight(weight)
    core_ids = list(range(N_CORES))

    # Launch A: per-core |W| partial sums -> alpha on host (8 adds of
    # device-computed partials; the reduction itself ran on device).
    res_a = run_bass_kernel_spmd(nc_a, [{"w": was[c]} for c in core_ids],
                                 core_ids, trace=trace)
    total = np.float64(0.0)
    for c in core_ids:
        total += np.float64(res_a.results[c]["s"].sum(dtype=np.float64))
    alpha = np.float32(total / (FAN_OUT * FAN_IN)) + np.float32(1e-8)

    x3 = x.reshape(-1, FAN_IN)[: n_mblk * MBLK].reshape(n_mblk, MBLK, FAN_IN)
    ia_v = np.full((128, 1), 1.0 / alpha, dtype=np.float32)
    a127_v = np.full((128, 1), alpha / np.float32(127.0), dtype=np.float32)
    in_maps = [{"x": x3, "wg": wgs[c], "inv_alpha": ia_v, "alpha127": a127_v}
               for c in core_ids]
    res_b = run_bass_kernel_spmd(nc_b, in_maps, core_ids, trace=trace)

    outs = [res_b.results[c]["out"].reshape(n_mblk * MBLK, FO_CORE)
            for c in core_ids]
    full = np.concatenate(outs, axis=1)
    exec_ns = None
    if trace and res_a.exec_time_ns is not None and res_b.exec_time_ns is not None:
        exec_ns = res_a.exec_time_ns + res_b.exec_time_ns
    return full, exec_ns


def kernel(x, weight):
    x = np.asarray(x)
    out2d, _ = run(x, weight, trace=False)
    return out2d.reshape(*x.shape[:-1], FAN_OUT).astype(np.float32)


# revision 7
# speedup vs baseline: 1.4465x; 1.0237x over previous
"""BitLinear (W1.58 / int8-activation quant) dense layer on 8 Trainium2 cores.

Math (matches the reference exactly up to fp rounding):
  alpha = mean(|W|) + 1e-8                     (global absmean, ternary weight scale)
  Wq    = clip(round(W/alpha), -1, 1)          (ternary, exact in bf16)
  gamma = clip(max|x_row|, 1e-8)               (per-token absmax)
  Qx    = round(x * 127/gamma)                 (integers in [-127,127], exact in bf16)
  out   = (Qx @ Wq.T) * (gamma * alpha / 127)

The integer matmul (bf16 operands, fp32 PSUM accumulation) is exact: products
are integers <= 127 and partial sums < 2^24.  All quantization, rounding and
the matmul run on device; the host only shards/reassembles and adds the eight
per-core |W| partial sums (launch A) into the scalar alpha.

Sharding: column-parallel (tensor-parallel on fan_out).  Each core owns a
2048-wide slice of the 16384 fan_out, x is replicated, no collectives.
"""

import sys
import types

import numpy as np

for _p in ("/opt/trn_rl_repo",):
    if _p not in sys.path:
        sys.path.insert(0, _p)

import concourse.bass as bass  # noqa: E402
import concourse.mybir as mybir  # noqa: E402
import concourse.tile as tile  # noqa: E402
from concourse import bacc  # noqa: E402
from concourse.bass_utils import run_bass_kernel_spmd  # noqa: E402

F32 = mybir.dt.float32
BF16 = mybir.dt.bfloat16
AX = mybir.AxisListType.X
ALU = mybir.AluOpType
ACT_FN = mybir.ActivationFunctionType

N_CORES = 8
FAN_IN = 4096
FAN_OUT = 16384
NTOK = 4 * 2048                     # 8192 tokens
FO_CORE = FAN_OUT // N_CORES        # 2048 fan_out columns per core
MBLK = 128                          # token block (PE partition dim)
N_MBLK = NTOK // MBLK               # 64 token blocks
KT = FAN_IN // 128                  # 32 contraction tiles of 128
NCH = FO_CORE // 512                # 4 psum chunks of 512
MAGIC = 12582912.0                  # 1.5 * 2**23: (v+MAGIC)-MAGIC == rint(v)


def _install_ntff_hook():
    """Register the axon NTFF profile hook trn_boot couldn't install
    (this image's antenv package lacks the axon_hooks submodule)."""
    if "antenv.axon_hooks" in sys.modules:
        return
    try:
        from trn_agent_boot.trn_boot import _ntff_profile_via_ctypes

        hook = _ntff_profile_via_ctypes("/opt/axon/libaxon_pjrt.so")
    except Exception:
        hook = None
    mod = types.ModuleType("antenv.axon_hooks")
    mod.get_axon_ntff_profile_hook = lambda: hook
    mod.set_axon_ntff_profile_hook = lambda h: None
    sys.modules["antenv.axon_hooks"] = mod
    import antenv

    antenv.axon_hooks = mod


def build_alpha_nc():
    """Launch A: per-core partial sum of |W| (per partition), for alpha."""
    nc = bacc.Bacc("TRN2", target_bir_lowering=False, debug=False,
                   num_devices=N_CORES)
    n_t = FO_CORE // MBLK  # 16 tiles of [128, 4096]
    w_in = nc.declare_dram_parameter("w", [n_t, MBLK, FAN_IN], F32, isOutput=False)
    s_out = nc.declare_dram_parameter("s", [MBLK, 1], F32, isOutput=True)
    with tile.TileContext(nc) as tc:
        with tc.tile_pool(name="io", bufs=3) as pool, \
             tc.tile_pool(name="acc", bufs=1) as accp:
            accv = accp.tile([MBLK, n_t], F32)
            for i in range(n_t):
                t = pool.tile([MBLK, FAN_IN], F32)
                nc.sync.dma_start(t[:], w_in[i])
                nc.vector.reduce_sum(accv[:, i:i + 1], t[:], axis=AX,
                                     apply_absolute_value=True)
            total = accp.tile([MBLK, 1], F32)
            nc.vector.reduce_sum(total[:], accv[:], axis=AX)
            nc.sync.dma_start(s_out[:], total[:])
    nc.compile()
    return nc


def build_main_nc(n_mblk=N_MBLK, nmm=512):
    """Launch B: quantize W (ternary) + x (int8 grid), exact bf16 matmul,
    per-token rescale.  One core's fan_out slice, all tokens.

    Schedule notes:
    - x loads + qx transposes ride the ACT HWDGE ring; the 32 MiB weight
      stream owns the SP ring so it is never stalled behind an x-tile
      WAR wait.
    - The x pipelines for m=0,1 are emitted before the weight stream so
      their DMAs/quant run during the weight DMA (PE warm-up food).
    - W-quant is a 3-engine pipeline (ACT round / DVE min / GpSimd max)
      so DVE keeps serving the x pipeline during the ramp.
    - The first two token blocks share one a-loop (8 PSUM banks) so the
      PE has 2 blocks of matmul work while weight tiles trickle in.
    """
    assert FO_CORE % nmm == 0
    nch = FO_CORE // nmm
    banks = nmm // 512
    nc = bacc.Bacc("TRN2", target_bir_lowering=False, debug=False,
                   num_devices=N_CORES)
    x_in = nc.declare_dram_parameter("x", [n_mblk, MBLK, FAN_IN], F32,
                                     isOutput=False)
    # wg[a, p, n] = W_slice.T[a*128 + p, n]  (k-tile a, k-within-tile p)
    wg_in = nc.declare_dram_parameter("wg", [KT, 128, FO_CORE], F32,
                                      isOutput=False)
    ia_in = nc.declare_dram_parameter("inv_alpha", [128, 1], F32, isOutput=False)
    a127_in = nc.declare_dram_parameter("alpha127", [128, 1], F32, isOutput=False)
    out_d = nc.declare_dram_parameter("out", [n_mblk, MBLK, FO_CORE], F32,
                                      isOutput=True)

    with tile.TileContext(nc) as tc:
        with tc.tile_pool(name="consts", bufs=1) as cpool, \
             tc.tile_pool(name="vec", bufs=4) as vpool, \
             tc.tile_pool(name="xio", bufs=2) as xpool, \
             tc.tile_pool(name="qxp", bufs=1) as qxpool, \
             tc.tile_pool(name="qxtp", bufs=2) as qxtpool, \
             tc.tile_pool(name="osbp", bufs=2) as opool, \
             tc.tile_pool(name="wfp", bufs=3) as wfpool, \
             tc.tile_pool(name="wqp", bufs=KT) as wqpool, \
             tc.tile_pool(name="ps", bufs=8 // banks, space="PSUM") as pspool:

            magic = cpool.tile([128, 1], F32)
            nc.gpsimd.memset(magic[:], MAGIC)
            negmagic = cpool.tile([128, 1], F32)
            nc.gpsimd.memset(negmagic[:], -MAGIC)
            ia = cpool.tile([128, 1], F32)
            nc.sync.dma_start(ia[:], ia_in[:])
            a127 = cpool.tile([128, 1], F32)
            nc.sync.dma_start(a127[:], a127_in[:])

            def x_pipeline(m, ring=None):
                """DMA+quantize+transpose one token block; returns (qxT, sc)."""
                ring = ring or nc.sync
                xh = []
                g2 = vpool.tile([128, 2], F32, tag="g2", name="g2")
                for h in range(2):
                    xt = xpool.tile([128, FAN_IN // 2], F32, tag="xt", name="xt")
                    ring.dma_start(xt[:], x_in[m, :, h * 2048:(h + 1) * 2048])
                    nc.vector.reduce_max(g2[:, h:h + 1], xt[:], axis=AX,
                                         apply_absolute_value=True)
                    xh.append(xt)
                g = vpool.tile([128, 1], F32, tag="g", name="g")
                nc.vector.reduce_max(g[:], g2[:], axis=AX)
                nc.vector.tensor_scalar_max(g[:], g[:], 1e-8)
                sc = vpool.tile([128, 1], F32, tag="sc", name="sc")
                nc.vector.tensor_scalar(out=sc[:], in0=g[:], scalar1=a127[:, 0:1],
                                        scalar2=None, op0=ALU.mult)
                rg = vpool.tile([128, 1], F32, tag="rg", name="rg")
                nc.vector.reciprocal(rg[:], g[:])
                nc.vector.tensor_scalar(out=rg[:], in0=rg[:], scalar1=127.0,
                                        scalar2=None, op0=ALU.mult)
                qx = qxpool.tile([128, FAN_IN], BF16, tag="qx", name="qx")
                for h in range(2):
                    # x*127/gamma + MAGIC (rounds to int), then -MAGIC -> bf16
                    nc.scalar.activation(xh[h][:], xh[h][:], ACT_FN.Identity,
                                         bias=magic[:, 0:1], scale=rg[:, 0:1])
                    nc.vector.tensor_scalar(out=qx[:, h * 2048:(h + 1) * 2048],
                                            in0=xh[h][:], scalar1=-MAGIC,
                                            scalar2=None, op0=ALU.add)
                # qxT[p, a, mm] = qx[mm, a*128 + p]
                qxT = qxtpool.tile([128, KT, 128], BF16, tag="qxT", name="qxT")
                nc.scalar.dma_start_transpose(qxT[:, :, :], qx[:, :])
                return qxT, sc

            def drain(psts, sc, m):
                """PSUM -> scaled f32 SBUF -> DRAM for one token block,
                in two half-width stores (halves osb SBUF footprint)."""
                hw = FO_CORE // 2
                per = hw // nmm
                for half in range(2):
                    osb = opool.tile([128, hw], F32, tag="osb", name="osb")
                    for j in range(per):
                        n = half * per + j
                        nc.scalar.activation(osb[:, j * nmm:(j + 1) * nmm],
                                             psts[n][:], ACT_FN.Copy,
                                             bias=0.0, scale=sc[:, 0:1])
                    nc.sync.dma_start(out_d[m, :, half * hw:(half + 1) * hw],
                                      osb[:])

            # ---- x pipelines for the first two blocks (run during W DMA).
            #      Block 0 loads lead the SP ring (ahead of the weight
            #      stream); block 1 loads ride the ACT ring so their WAR
            #      waits never stall the weight stream.
            head = []
            if n_mblk > 0:
                head.append(x_pipeline(0, ring=nc.sync))
            if n_mblk > 1:
                head.append(x_pipeline(1, ring=nc.scalar))

            # ---- Phase W: ternary-quantize the weight slice (resident bf16)
            wq_tiles = []
            for a in range(KT):
                wf = wfpool.tile([128, FO_CORE], F32, tag="wf", name="wf")
                nc.sync.dma_start(wf[:], wg_in[a])
                # wf = w*inv_alpha + MAGIC  (rounds to nearest-even integer)
                nc.vector.tensor_scalar(out=wf[:], in0=wf[:],
                                        scalar1=ia[:, 0:1], scalar2=MAGIC,
                                        op0=ALU.mult, op1=ALU.add)
                wq = wqpool.tile([128, FO_CORE], BF16, tag="wq", name="wq")
                # sign(rint(w/alpha)) == clip(rint(w/alpha), -1, 1): ternary
                nc.scalar.activation(wq[:], wf[:], ACT_FN.Sign,
                                     bias=negmagic[:, 0:1])
                wq_tiles.append(wq)

            # ---- Ramp: first two blocks share one a-loop (PE eats 2 blocks
            #      of matmuls while wq tiles arrive)
            nhead = len(head)
            psts = [[pspool.tile([128, nmm], F32, tag="ps", name="ps")
                     for _ in range(nch)] for _ in range(nhead)]
            for a in range(KT):
                for mb in range(nhead):
                    lt = head[mb][0][:, a, :]
                    for n in range(nch):
                        nc.tensor.matmul(psts[mb][n][:], lt,
                                         wq_tiles[a][:, n * nmm:(n + 1) * nmm],
                                         start=(a == 0), stop=(a == KT - 1))
            for mb in range(nhead):
                drain(psts[mb], head[mb][1], mb)

            # ---- Steady loop
            for m in range(nhead, n_mblk):
                qxT, sc = x_pipeline(m)
                ps1 = [pspool.tile([128, nmm], F32, tag="ps", name="ps")
                       for _ in range(nch)]
                for a in range(KT):
                    lt = qxT[:, a, :]
                    for n in range(nch):
                        nc.tensor.matmul(ps1[n][:], lt,
                                         wq_tiles[a][:, n * nmm:(n + 1) * nmm],
                                         start=(a == 0), stop=(a == KT - 1))
                drain(ps1, sc, m)
    nc.compile()
    return nc


_CACHE = {}


def _get_ncs(n_mblk=N_MBLK):
    key = ("ncs", n_mblk)
    if key not in _CACHE:
        _install_ntff_hook()
        _CACHE[key] = (build_alpha_nc(), build_main_nc(n_mblk))
    return _CACHE[key]


def _shard_weight(weight):
    """Per-core host-side layout prep (pure slicing/reshape glue)."""
    wgs, was = [], []
    for c in range(N_CORES):
        w_c = weight[c * FO_CORE:(c + 1) * FO_CORE, :]          # [2048, 4096]
        was.append(np.ascontiguousarray(w_c).reshape(FO_CORE // MBLK, MBLK,
                                                     FAN_IN))
        wg = np.ascontiguousarray(w_c.T).reshape(KT, 128, FO_CORE)
        wgs.append(wg)
    return wgs, was


def run(x, weight, trace=False, n_mblk=N_MBLK):
    """Returns (out, exec_ns) where exec_ns is summed HW time of both
    launches (None unless trace=True)."""
    x = np.ascontiguousarray(np.asarray(x, dtype=np.float32))
    weight = np.ascontiguousarray(np.asarray(weight, dtype=np.float32))
    nc_a, nc_b = _get_ncs(n_mblk)
    wgs, was = _shard_weight(weight)
    core_ids = list(range(N_CORES))

    # Launch A: per-core |W| partial sums -> alpha on host (8 adds of
    # device-computed partials; the reduction itself ran on device).
    res_a = run_bass_kernel_spmd(nc_a, [{"w": was[c]} for c in core_ids],
                                 core_ids, trace=trace)
    total = np.float64(0.0)
    for c in core_ids:
        total += np.float64(res_a.results[c]["s"].sum(dtype=np.float64))
    alpha = np.float32(total / (FAN_OUT * FAN_IN)) + np.float32(1e-8)

    x3 = x.reshape(-1, FAN_IN)[: n_mblk * MBLK].reshape(n_mblk, MBLK, FAN_IN)
    ia_v = np.full((128, 1), 1.0 / alpha, dtype=np.float32)
    a127_v = np.full((128, 1), alpha / np.float32(127.0), dtype=np.float32)
    in_maps = [{"x": x3, "wg": wgs[c], "inv_alpha": ia_v, "alpha127": a127_v}
               for c in core_ids]
    res_b = run_bass_kernel_spmd(nc_b, in_maps, core_ids, trace=trace)

    outs = [res_b.results[c]["out"].reshape(n_mblk * MBLK, FO_CORE)
            for c in core_ids]
    full = np.concatenate(outs, axis=1)
    exec_ns = None
    if trace and res_a.exec_time_ns is not None and res_b.exec_time_ns is not None:
        exec_ns = res_a.exec_time_ns + res_b.exec_time_ns
    return full, exec_ns


def kernel(x, weight):
    x = np.asarray(x)
    out2d, _ = run(x, weight, trace=False)
    return out2d.reshape(*x.shape[:-1], FAN_OUT).astype(np.float32)


# revision 8
# speedup vs baseline: 1.4582x; 1.0080x over previous
"""BitLinear (W1.58 / int8-activation quant) dense layer on 8 Trainium2 cores.

Math (matches the reference exactly up to fp rounding):
  alpha = mean(|W|) + 1e-8                     (global absmean, ternary weight scale)
  Wq    = clip(round(W/alpha), -1, 1)          (ternary, exact in bf16)
  gamma = clip(max|x_row|, 1e-8)               (per-token absmax)
  Qx    = round(x * 127/gamma)                 (integers in [-127,127], exact in bf16)
  out   = (Qx @ Wq.T) * (gamma * alpha / 127)

The integer matmul (bf16 operands, fp32 PSUM accumulation) is exact: products
are integers <= 127 and partial sums < 2^24.  All quantization, rounding and
the matmul run on device; the host only shards/reassembles and adds the eight
per-core |W| partial sums (launch A) into the scalar alpha.

Sharding: column-parallel (tensor-parallel on fan_out).  Each core owns a
2048-wide slice of the 16384 fan_out, x is replicated, no collectives.
"""

import sys
import types

import numpy as np

for _p in ("/opt/trn_rl_repo",):
    if _p not in sys.path:
        sys.path.insert(0, _p)

import concourse.bass as bass  # noqa: E402
import concourse.mybir as mybir  # noqa: E402
import concourse.tile as tile  # noqa: E402
from concourse import bacc  # noqa: E402
from concourse.bass_utils import run_bass_kernel_spmd  # noqa: E402

F32 = mybir.dt.float32
BF16 = mybir.dt.bfloat16
AX = mybir.AxisListType.X
ALU = mybir.AluOpType
ACT_FN = mybir.ActivationFunctionType

N_CORES = 8
FAN_IN = 4096
FAN_OUT = 16384
NTOK = 4 * 2048                     # 8192 tokens
FO_CORE = FAN_OUT // N_CORES        # 2048 fan_out columns per core
MBLK = 128                          # token block (PE partition dim)
N_MBLK = NTOK // MBLK               # 64 token blocks
KT = FAN_IN // 128                  # 32 contraction tiles of 128
NCH = FO_CORE // 512                # 4 psum chunks of 512
MAGIC = 12582912.0                  # 1.5 * 2**23: (v+MAGIC)-MAGIC == rint(v)


def _install_ntff_hook():
    """Register the axon NTFF profile hook trn_boot couldn't install
    (this image's antenv package lacks the axon_hooks submodule)."""
    if "antenv.axon_hooks" in sys.modules:
        return
    try:
        from trn_agent_boot.trn_boot import _ntff_profile_via_ctypes

        hook = _ntff_profile_via_ctypes("/opt/axon/libaxon_pjrt.so")
    except Exception:
        hook = None
    mod = types.ModuleType("antenv.axon_hooks")
    mod.get_axon_ntff_profile_hook = lambda: hook
    mod.set_axon_ntff_profile_hook = lambda h: None
    sys.modules["antenv.axon_hooks"] = mod
    import antenv

    antenv.axon_hooks = mod


def build_alpha_nc():
    """Launch A: per-core partial sum of |W| (per partition), for alpha."""
    nc = bacc.Bacc("TRN2", target_bir_lowering=False, debug=False,
                   num_devices=N_CORES)
    n_t = FO_CORE // MBLK  # 16 tiles of [128, 4096]
    w_in = nc.declare_dram_parameter("w", [n_t, MBLK, FAN_IN], F32, isOutput=False)
    s_out = nc.declare_dram_parameter("s", [MBLK, 1], F32, isOutput=True)
    with tile.TileContext(nc) as tc:
        with tc.tile_pool(name="io", bufs=3) as pool, \
             tc.tile_pool(name="acc", bufs=1) as accp:
            accv = accp.tile([MBLK, n_t], F32)
            for i in range(n_t):
                t = pool.tile([MBLK, FAN_IN], F32)
                nc.sync.dma_start(t[:], w_in[i])
                nc.vector.reduce_sum(accv[:, i:i + 1], t[:], axis=AX,
                                     apply_absolute_value=True)
            total = accp.tile([MBLK, 1], F32)
            nc.vector.reduce_sum(total[:], accv[:], axis=AX)
            nc.sync.dma_start(s_out[:], total[:])
    nc.compile()
    return nc


def build_main_nc(n_mblk=N_MBLK, nmm=512):
    """Launch B: quantize W (ternary) + x (int8 grid), exact bf16 matmul,
    per-token rescale.  One core's fan_out slice, all tokens.

    Schedule notes:
    - x loads + qx transposes ride the ACT HWDGE ring; the 32 MiB weight
      stream owns the SP ring so it is never stalled behind an x-tile
      WAR wait.
    - The x pipelines for m=0,1 are emitted before the weight stream so
      their DMAs/quant run during the weight DMA (PE warm-up food).
    - W-quant is a 3-engine pipeline (ACT round / DVE min / GpSimd max)
      so DVE keeps serving the x pipeline during the ramp.
    - The first two token blocks share one a-loop (8 PSUM banks) so the
      PE has 2 blocks of matmul work while weight tiles trickle in.
    """
    assert FO_CORE % nmm == 0
    nch = FO_CORE // nmm
    banks = nmm // 512
    nc = bacc.Bacc("TRN2", target_bir_lowering=False, debug=False,
                   num_devices=N_CORES)
    x_in = nc.declare_dram_parameter("x", [n_mblk, MBLK, FAN_IN], F32,
                                     isOutput=False)
    # wg[a, p, n] = W_slice.T[a*128 + p, n]  (k-tile a, k-within-tile p)
    wg_in = nc.declare_dram_parameter("wg", [KT, 128, FO_CORE], F32,
                                      isOutput=False)
    ia_in = nc.declare_dram_parameter("inv_alpha", [128, 1], F32, isOutput=False)
    a127_in = nc.declare_dram_parameter("alpha127", [128, 1], F32, isOutput=False)
    out_d = nc.declare_dram_parameter("out", [n_mblk, MBLK, FO_CORE], F32,
                                      isOutput=True)

    with tile.TileContext(nc) as tc:
        with tc.tile_pool(name="consts", bufs=1) as cpool, \
             tc.tile_pool(name="vec", bufs=4) as vpool, \
             tc.tile_pool(name="xio", bufs=2) as xpool, \
             tc.tile_pool(name="qxp", bufs=1) as qxpool, \
             tc.tile_pool(name="qxtp", bufs=2) as qxtpool, \
             tc.tile_pool(name="osbp", bufs=1) as opool, \
             tc.tile_pool(name="wfp", bufs=4) as wfpool, \
             tc.tile_pool(name="wqp", bufs=KT) as wqpool, \
             tc.tile_pool(name="ps", bufs=8 // banks, space="PSUM") as pspool:

            magic = cpool.tile([128, 1], F32)
            nc.gpsimd.memset(magic[:], MAGIC)
            negmagic = cpool.tile([128, 1], F32)
            nc.gpsimd.memset(negmagic[:], -MAGIC)
            ia = cpool.tile([128, 1], F32)
            nc.sync.dma_start(ia[:], ia_in[:])
            a127 = cpool.tile([128, 1], F32)
            nc.sync.dma_start(a127[:], a127_in[:])

            def x_pipeline(m, ring=None):
                """DMA+quantize+transpose one token block; returns (qxT, sc)."""
                ring = ring or nc.sync
                xh = []
                g2 = vpool.tile([128, 2], F32, tag="g2", name="g2")
                for h in range(2):
                    xt = xpool.tile([128, FAN_IN // 2], F32, tag="xt", name="xt")
                    ring.dma_start(xt[:], x_in[m, :, h * 2048:(h + 1) * 2048])
                    nc.vector.reduce_max(g2[:, h:h + 1], xt[:], axis=AX,
                                         apply_absolute_value=True)
                    xh.append(xt)
                g = vpool.tile([128, 1], F32, tag="g", name="g")
                nc.vector.reduce_max(g[:], g2[:], axis=AX)
                nc.vector.tensor_scalar_max(g[:], g[:], 1e-8)
                sc = vpool.tile([128, 1], F32, tag="sc", name="sc")
                nc.vector.tensor_scalar(out=sc[:], in0=g[:], scalar1=a127[:, 0:1],
                                        scalar2=None, op0=ALU.mult)
                rg = vpool.tile([128, 1], F32, tag="rg", name="rg")
                nc.vector.reciprocal(rg[:], g[:])
                nc.vector.tensor_scalar(out=rg[:], in0=rg[:], scalar1=127.0,
                                        scalar2=None, op0=ALU.mult)
                qx = qxpool.tile([128, FAN_IN], BF16, tag="qx", name="qx")
                for h in range(2):
                    # x*127/gamma + MAGIC (rounds to int), then -MAGIC -> bf16
                    nc.scalar.activation(xh[h][:], xh[h][:], ACT_FN.Identity,
                                         bias=magic[:, 0:1], scale=rg[:, 0:1])
                    nc.vector.tensor_scalar(out=qx[:, h * 2048:(h + 1) * 2048],
                                            in0=xh[h][:], scalar1=-MAGIC,
                                            scalar2=None, op0=ALU.add)
                # qxT[p, a, mm] = qx[mm, a*128 + p]
                qxT = qxtpool.tile([128, KT, 128], BF16, tag="qxT", name="qxT")
                nc.scalar.dma_start_transpose(qxT[:, :, :], qx[:, :])
                return qxT, sc

            def drain(psts, sc, m):
                """PSUM -> scaled f32 SBUF -> DRAM for one token block,
                in two half-width stores (halves osb SBUF footprint)."""
                hw = FO_CORE // 2
                per = hw // nmm
                for half in range(2):
                    osb = opool.tile([128, hw], F32, tag="osb", name="osb")
                    for j in range(per):
                        n = half * per + j
                        nc.scalar.activation(osb[:, j * nmm:(j + 1) * nmm],
                                             psts[n][:], ACT_FN.Copy,
                                             bias=0.0, scale=sc[:, 0:1])
                    nc.sync.dma_start(out_d[m, :, half * hw:(half + 1) * hw],
                                      osb[:])

            # ---- x pipelines for the first two blocks (run during W DMA).
            #      Block 0 loads lead the SP ring (ahead of the weight
            #      stream); block 1 loads ride the ACT ring so their WAR
            #      waits never stall the weight stream.
            head = []
            if n_mblk > 0:
                head.append(x_pipeline(0, ring=nc.sync))
            if n_mblk > 1:
                head.append(x_pipeline(1, ring=nc.scalar))

            # ---- Phase W: ternary-quantize the weight slice (resident bf16)
            wq_tiles = []
            for a in range(KT):
                wf = wfpool.tile([128, FO_CORE], F32, tag="wf", name="wf")
                nc.sync.dma_start(wf[:], wg_in[a])
                # wf = w*inv_alpha + MAGIC  (rounds to nearest-even integer)
                nc.vector.tensor_scalar(out=wf[:], in0=wf[:],
                                        scalar1=ia[:, 0:1], scalar2=MAGIC,
                                        op0=ALU.mult, op1=ALU.add)
                wq = wqpool.tile([128, FO_CORE], BF16, tag="wq", name="wq")
                # sign(rint(w/alpha)) == clip(rint(w/alpha), -1, 1): ternary
                nc.scalar.activation(wq[:], wf[:], ACT_FN.Sign,
                                     bias=negmagic[:, 0:1])
                wq_tiles.append(wq)

            # ---- Ramp: first two blocks share one a-loop (PE eats 2 blocks
            #      of matmuls while wq tiles arrive)
            nhead = len(head)
            psts = [[pspool.tile([128, nmm], F32, tag="ps", name="ps")
                     for _ in range(nch)] for _ in range(nhead)]
            for a in range(KT):
                for mb in range(nhead):
                    lt = head[mb][0][:, a, :]
                    for n in range(nch):
                        nc.tensor.matmul(psts[mb][n][:], lt,
                                         wq_tiles[a][:, n * nmm:(n + 1) * nmm],
                                         start=(a == 0), stop=(a == KT - 1))
            for mb in range(nhead):
                drain(psts[mb], head[mb][1], mb)

            # ---- Steady loop
            for m in range(nhead, n_mblk):
                qxT, sc = x_pipeline(m)
                ps1 = [pspool.tile([128, nmm], F32, tag="ps", name="ps")
                       for _ in range(nch)]
                for a in range(KT):
                    lt = qxT[:, a, :]
                    for n in range(nch):
                        nc.tensor.matmul(ps1[n][:], lt,
                                         wq_tiles[a][:, n * nmm:(n + 1) * nmm],
                                         start=(a == 0), stop=(a == KT - 1))
                drain(ps1, sc, m)
    nc.compile()
    return nc


_CACHE = {}


def _get_ncs(n_mblk=N_MBLK):
    key = ("ncs", n_mblk)
    if key not in _CACHE:
        _install_ntff_hook()
        _CACHE[key] = (build_alpha_nc(), build_main_nc(n_mblk))
    return _CACHE[key]


def _shard_weight(weight):
    """Per-core host-side layout prep (pure slicing/reshape glue)."""
    wgs, was = [], []
    for c in range(N_CORES):
        w_c = weight[c * FO_CORE:(c + 1) * FO_CORE, :]          # [2048, 4096]
        was.append(np.ascontiguousarray(w_c).reshape(FO_CORE // MBLK, MBLK,
                                                     FAN_IN))
        wg = np.ascontiguousarray(w_c.T).reshape(KT, 128, FO_CORE)
        wgs.append(wg)
    return wgs, was


def run(x, weight, trace=False, n_mblk=N_MBLK):
    """Returns (out, exec_ns) where exec_ns is summed HW time of both
    launches (None unless trace=True)."""
    x = np.ascontiguousarray(np.asarray(x, dtype=np.float32))
    weight = np.ascontiguousarray(np.asarray(weight, dtype=np.float32))
    nc_a, nc_b = _get_ncs(n_mblk)
    wgs, was = _shard_weight(weight)
    core_ids = list(range(N_CORES))

    # Launch A: per-core |W| partial sums -> alpha on host (8 adds of
    # device-computed partials; the reduction itself ran on device).
    res_a = run_bass_kernel_spmd(nc_a, [{"w": was[c]} for c in core_ids],
                                 core_ids, trace=trace)
    total = np.float64(0.0)
    for c in core_ids:
        total += np.float64(res_a.results[c]["s"].sum(dtype=np.float64))
    alpha = np.float32(total / (FAN_OUT * FAN_IN)) + np.float32(1e-8)

    x3 = x.reshape(-1, FAN_IN)[: n_mblk * MBLK].reshape(n_mblk, MBLK, FAN_IN)
    ia_v = np.full((128, 1), 1.0 / alpha, dtype=np.float32)
    a127_v = np.full((128, 1), alpha / np.float32(127.0), dtype=np.float32)
    in_maps = [{"x": x3, "wg": wgs[c], "inv_alpha": ia_v, "alpha127": a127_v}
               for c in core_ids]
    res_b = run_bass_kernel_spmd(nc_b, in_maps, core_ids, trace=trace)

    outs = [res_b.results[c]["out"].reshape(n_mblk * MBLK, FO_CORE)
            for c in core_ids]
    full = np.concatenate(outs, axis=1)
    exec_ns = None
    if trace and res_a.exec_time_ns is not None and res_b.exec_time_ns is not None:
        exec_ns = res_a.exec_time_ns + res_b.exec_time_ns
    return full, exec_ns


def kernel(x, weight):
    x = np.asarray(x)
    out2d, _ = run(x, weight, trace=False)
    return out2d.reshape(*x.shape[:-1], FAN_OUT).astype(np.float32)
